# revision 6
# baseline (speedup 1.0000x reference)
"""Trainium2 Bass kernel for nn_DiagonalLayer (per-gene weighted feature sum).

out[b, g] = sum_f x[b, 3g+f] * w[3g+f] + bias[g]

Sharding: data-parallel over the batch dim — 4096 rows split as 512 rows on
each of the 8 NeuronCores; w/bias replicated (tiny). Output gathered by
concatenation along batch.

Self-contained: hardcodes shapes; only imports the concourse toolchain.
"""

import sys

import numpy as np

if "/opt/trn_rl_repo" not in sys.path:
    sys.path.insert(0, "/opt/trn_rl_repo")

B = 4096
GF = 27687
G = 9229
F = 3
NCORES = 8
BSH = B // NCORES  # 512 batch rows per core
PT = 128  # SBUF partitions
NT = BSH // PT  # 4 batch tiles per core
GC = 2308  # genes per chunk -> chunks of 2308,2308,2308,2305

_cached_nc = None


def _gene_chunks():
    chunks = []
    c0 = 0
    while c0 < G:
        gc = min(GC, G - c0)
        chunks.append((c0, gc))
        c0 += gc
    return chunks


def _build_nc():
    import concourse.bacc as bacc
    import concourse.mybir as mybir
    import concourse.tile as tile

    f32 = mybir.dt.float32
    nc = bacc.Bacc(
        "TRN2", target_bir_lowering=False, debug=False, num_devices=NCORES
    )
    x = nc.dram_tensor("x", [BSH, GF], f32, kind="ExternalInput")
    w = nc.dram_tensor("w", [GF], f32, kind="ExternalInput")
    bias = nc.dram_tensor("bias", [G], f32, kind="ExternalInput")
    out = nc.dram_tensor("out", [BSH, G], f32, kind="ExternalOutput")

    with tile.TileContext(nc) as tc:
        with (
            tc.tile_pool(name="wb", bufs=2) as wb_pool,
            tc.tile_pool(name="bb", bufs=2) as bb_pool,
            tc.tile_pool(name="xc", bufs=3) as x_pool,
            tc.tile_pool(name="oc", bufs=3) as o_pool,
        ):
            for c0, gc in _gene_chunks():
                wbt = wb_pool.tile([PT, F * gc], f32, tag="wb")
                nc.sync.dma_start(
                    out=wbt[:1, :], in_=w[None, F * c0 : F * (c0 + gc)]
                )
                nc.gpsimd.partition_broadcast(wbt[:, :], wbt[:1, :])

                bbt = bb_pool.tile([PT, gc], f32, tag="bb")
                nc.sync.dma_start(out=bbt[:1, :], in_=bias[None, c0 : c0 + gc])
                nc.gpsimd.partition_broadcast(bbt[:, :], bbt[:1, :])

                for t in range(NT):
                    xc = x_pool.tile([PT, F * gc], f32, tag="xc")
                    nc.sync.dma_start(
                        out=xc[:, :],
                        in_=x[t * PT : (t + 1) * PT, F * c0 : F * (c0 + gc)],
                    )
                    nc.vector.tensor_mul(xc[:, :], xc[:, :], wbt[:, :])
                    oc = o_pool.tile([PT, gc], f32, tag="oc")
                    x3 = xc[:, :].rearrange("p (g f) -> p g f", f=F)
                    nc.vector.reduce_sum(oc[:, :], x3, axis=mybir.AxisListType.X)
                    nc.vector.tensor_add(oc[:, :], oc[:, :], bbt[:, :])
                    nc.sync.dma_start(
                        out=out[t * PT : (t + 1) * PT, c0 : c0 + gc], in_=oc[:, :]
                    )
    if not nc.is_finalized():
        nc.finalize()
    return nc


def _get_nc():
    global _cached_nc
    if _cached_nc is None:
        _cached_nc = _build_nc()
    return _cached_nc


def run(x, weights, bias, trace=False, tmpdir=None):
    from concourse.bass_utils import run_bass_kernel_spmd

    x = np.ascontiguousarray(np.asarray(x, dtype=np.float32))
    weights = np.ascontiguousarray(np.asarray(weights, dtype=np.float32))
    bias_np = np.ascontiguousarray(np.asarray(bias, dtype=np.float32))

    nc = _get_nc()
    in_maps = [
        {
            "x": np.ascontiguousarray(x[c * BSH : (c + 1) * BSH]),
            "w": weights,
            "bias": bias_np,
        }
        for c in range(NCORES)
    ]
    res = run_bass_kernel_spmd(
        nc, in_maps, list(range(NCORES)), trace=trace, tmpdir=tmpdir
    )
    outs = [res.results[c]["out"] for c in range(NCORES)]
    full = np.concatenate(outs, axis=0)
    return full, res


def kernel(x, weights, bias):
    full, _ = run(x, weights, bias, trace=False)
    return full
